# revision 1
# baseline (speedup 1.0000x reference)
"""Trainium2 Bass kernel for nn_MultiHeadAttention_8667244003725.

B=4, S=1024, E=1024, H=16, D=64.  Reference:
  q/k/v = einsum('bse,hed->bhsd', x, W{q,k,v})
  scores = q@k^T/sqrt(D), causal mask, softmax
  heads -> concat (B,S,E);  out = W_O @ concat  (contracts over SEQUENCE dim)
  returns (B, E, E).

Sharding: 8 cores = 4 batches x 2 head-groups (8 heads each).  Because the
output projection contracts over the sequence dim, sharding heads shards the
output columns: core c computes out[b, :, 512*g : 512*g+512] with b=c//2,
g=c%2.  No collectives.

Per-core pipeline (all matmuls in float32r, 1 cyc/row at N>=256):
  xT    = x[b]^T via PE transposes                  [e, s]
  QT/KT = (Wq|Wk pair)^T @ xT   packed 2 heads/matmul -> [2*64, s]
  V_all = xT^T @ Wv_all  -> natural [s, 8*64], stored [s, 8*(64+1)] with a
          ones column per head (row-sum trick)
  per head, per 512-wide q chunk:
     ST[k,q] = KT^T-block @ QT  (PSUM), +causal mask on diagonal blocks,
     P = exp(ST/8)  (no max subtraction; scores/8 <= ~6, fp32-safe),
     OT[65,q] += [V_k|1]^T @ P  accumulated over k blocks
  OT -> transpose 128-col blocks back to q-major [q, 64|l], divide by l,
  write into C[s, 512];  out-slice = W_O^T-chunks^T @ C chunks.
"""

import sys

if '/opt/trn_rl_repo' not in sys.path:
    sys.path.insert(0, '/opt/trn_rl_repo')

import numpy as np

import concourse.bass as bass
import concourse.mybir as mybir
import concourse.tile as tile
from concourse.masks import make_identity

F32 = mybir.dt.float32
F32R = mybir.dt.float32r
AF = mybir.ActivationFunctionType

S = 1024          # sequence
E = 1024          # embed
D = 64            # head dim
HC = 8            # heads per core
NO = 512          # output columns per core
NEG = -1.0e30


def _split_sync_waits(nc, limit=1):
    """The walrus build in this env rejects >1 sem-wait per instruction.
    Hoist excess waits onto preceding same-engine no-ops (same queue, so
    program order preserves the wait semantics)."""
    n = 0
    for f in nc.m.functions:
        for bb in f.blocks:
            out = []
            for ins in bb.instructions:
                si = ins.sync_info
                waits = list(si.on_wait) if si is not None else []
                if len(waits) > limit:
                    excess, keep = waits[:-limit], waits[-limit:]
                    for i in range(0, len(excess), limit):
                        grp = excess[i:i + limit]
                        n += 1
                        out.append(mybir.InstNoOp(
                            name=f'I-synsplit-{n}', ins=[], outs=[],
                            engine=ins.engine,
                            sync_info=mybir.SyncInfo(on_wait=list(grp),
                                                     on_update=[])))
                    si.on_wait = keep
                out.append(ins)
            bb.instructions = out
    return n


def build_nc(split_waits=True):
    nc = bass.Bass()
    BF = mybir.dt.bfloat16
    xb = nc.dram_tensor('xb', [E, S], BF, kind='ExternalInput')   # x[b]^T
    wq = nc.dram_tensor('wq', [E, HC * D], BF, kind='ExternalInput')
    wk = nc.dram_tensor('wk', [E, HC * D], BF, kind='ExternalInput')
    wv = nc.dram_tensor('wv', [E, HC * D], BF, kind='ExternalInput')
    wo = nc.dram_tensor('wo', [E, E], BF, kind='ExternalInput')   # W_O^T
    out = nc.dram_tensor('out', [E, NO], F32, kind='ExternalOutput')

    with tile.TileContext(nc) as tc:
        _emit(nc, tc, xb, wq, wk, wv, wo, out)
    if split_waits:
        _split_sync_waits(nc)
    return nc


def _emit(nc, tc, xb, wq, wk, wv, wo, out):
    BF = mybir.dt.bfloat16

    with (
        tc.tile_pool(name='const', bufs=1) as constp,
        tc.tile_pool(name='bigT', bufs=2) as bigT,      # xTall + WOTall
        tc.tile_pool(name='qk', bufs=1) as qkp,
        tc.tile_pool(name='vall', bufs=1) as vallp,
        tc.tile_pool(name='cbuf', bufs=1) as cp,
        tc.tile_pool(name='psA', bufs=2, space='PSUM') as psA,   # [128,512] mm
        tc.tile_pool(name='psB', bufs=2, space='PSUM') as psB,   # ot accum
        tc.tile_pool(name='psC', bufs=2, space='PSUM') as psC,   # transposes
    ):
        # ---- constants ----------------------------------------------------
        ident = constp.tile([128, 128], BF, tag='ident')
        make_identity(nc, ident[:])
        identf = constp.tile([128, 128], F32, tag='identf')
        make_identity(nc, identf[:])
        ones8 = constp.tile([128, 8], BF, tag='ones8')
        nc.gpsimd.memset(ones8[:], 1.0)
        # multiplicative causal mask for the [128,128] diagonal corner:
        # tri[k, q] = 1 where q >= k else 0
        tri = constp.tile([128, 128], BF, tag='tri')
        nc.gpsimd.memset(tri[:], 1.0)
        nc.gpsimd.affine_select(
            out=tri[:], in_=tri[:], compare_op=mybir.AluOpType.is_ge,
            fill=0.0, base=0, channel_multiplier=-1, pattern=[[1, 128]])

        # warm the ACT exp table while DMAs run
        warm = constp.tile([1, 2], F32, tag='warm')
        nc.scalar.activation(warm[:], ones8[0:1, 0:2], AF.Exp, scale=0.125)

        # ---- xT (bf16, host-transposed + pre-cast): per-chunk DMAs so
        # consumers trickle-start as each chunk lands
        xTall = bigT.tile([128, 8 * S], BF, tag='bigT', name='xTall')
        for ec in range(8):
            nc.sync.dma_start(xTall[:, ec * S:(ec + 1) * S],
                              xb[ec * 128:(ec + 1) * 128, :])
        xT = [xTall[:, ec * S:(ec + 1) * S] for ec in range(8)]

        wpool = tc.tile_pool(name='wts', bufs=1)
        wp = wpool.__enter__()
        # ---- weights (cast to bf16) --------------------------------------
        wqt, wkt, wvt = [], [], []
        for qi, (lst, src, nm) in enumerate(((wqt, wq, 'wq'), (wkt, wk, 'wk'),
                                             (wvt, wv, 'wv'))):
            wall = wp.tile([128, 8 * HC * D], BF, tag=f'{nm}all',
                           name=f'{nm}all')
            for ec in range(8):
                sl = wall[:, ec * HC * D:(ec + 1) * HC * D]
                nc.scalar.dma_start(sl, src[ec * 128:(ec + 1) * 128, :])
                lst.append(sl)

        # ---- QKV ----------------------------------------------------------
        # QT/KT packed head pairs: QT2[p][0:64,:] = head 2p, [64:128,:] = 2p+1
        QT2 = [qkp.tile([128, S], BF, tag=f'q{p}', name=f'QT2_{p}')
               for p in range(4)]
        KT2 = [qkp.tile([128, S], BF, tag=f'k{p}', name=f'KT2_{p}')
               for p in range(4)]
        # Q jobs first (all use wq, which is DMA'd first), then K jobs.
        # ec-major across 4 concurrent psum groups so matmuls trickle in as
        # each weight chunk arrives instead of stalling per-job.
        qkv_jobs = [(QT2[p], wqt, p) for p in range(4)] + \
                   [(KT2[p], wkt, p) for p in range(4)]
        for base in range(0, len(qkv_jobs), 2):
            chunk = qkv_jobs[base:base + 2]
            pss = {}
            for ci, (dst, wt, p) in enumerate(chunk):
                pool_, tag_ = (psA, 'mm') if ci == 0 else (psB, 'ot')
                pss[ci] = [pool_.tile([128, 512], F32, tag=tag_,
                                      name=f'qk_{base + ci}_{sc}')
                           for sc in range(2)]
            for ec in range(8):
                for ci, (dst, wt, p) in enumerate(chunk):
                    for sc in range(2):
                        nc.tensor.matmul(
                            pss[ci][sc][:],
                            wt[ec][:, p * 128:(p + 1) * 128],
                            xT[ec][:, sc * 512:(sc + 1) * 512],
                            start=(ec == 0), stop=(ec == 7))
            for ci, (dst, wt, p) in enumerate(chunk):
                for sc in range(2):
                    nc.vector.tensor_copy(dst[:, sc * 512:(sc + 1) * 512],
                                          pss[ci][sc][:])

        # V natural [s, 8*(64+1)] bf16: per head 64 value cols + a ones col
        Vall = [vallp.tile([128, HC * (D + 1)], BF, tag=f'v{st}',
                           name=f'Vall{st}') for st in range(8)]
        for st in range(8):
            ps = psA.tile([128, 512], F32, tag='mm')
            for ec in range(8):
                nc.tensor.matmul(ps[:],
                                 xT[ec][:, st * 128:(st + 1) * 128],
                                 wvt[ec],
                                 start=(ec == 0), stop=(ec == 7))
            v3 = Vall[st][:].rearrange('p (h d) -> p h d', h=HC)
            nc.vector.tensor_copy(v3[:, :, 0:D],
                                  ps[:].rearrange('p (h d) -> p h d', h=HC))
            nc.vector.tensor_copy(v3[:, :, D:D + 1],
                                  ones8[:].rearrange('p (h o) -> p h o', o=1))
        wpool.__exit__(None, None, None)

        # ---- attention + C ------------------------------------------------
        C = [cp.tile([128, NO], BF, tag=f'c{st}', name=f'C{st}')
             for st in range(8)]
        apool = tc.tile_pool(name='attn', bufs=4)
        sstr = apool.__enter__()
        for p in range(4):
            heads = (2 * p, 2 * p + 1)
            QTh = {h: QT2[p][64 * (h % 2):64 * (h % 2) + 64, :] for h in heads}
            KTh = {h: KT2[p][64 * (h % 2):64 * (h % 2) + 64, :] for h in heads}
            for qc in range(2):
                nkb = 4 * qc + 4
                ots_ = {h: psB.tile([128, 512], F32, tag='ot',
                                    name=f'ot_{h}_{qc}') for h in heads}
                for t in range(0, nkb, 2):
                    kbs = (t, t + 1)
                    # per head: both kb score blocks into one 2-bank psum
                    # tile, one exp over the pair, two OT accumulations
                    for h in heads:
                        stp = psA.tile([128, 1024], F32, tag='mm',
                                       name=f'stp_{h}_{qc}_{t}')
                        pexp = sstr.tile([128, 1024], BF, tag='pexp',
                                         name=f'pexp_{h}_{qc}_{t}')
                        offs = []
                        for sl, kb in enumerate(kbs):
                            j = kb - 4 * qc
                            off = 128 * j if j >= 0 else 0
                            W = 512 - off
                            offs.append((sl, kb, j, off, W))
                            nc.tensor.matmul(
                                stp[:128, sl * 512:sl * 512 + W],
                                KTh[h][:, kb * 128:(kb + 1) * 128],
                                QTh[h][:, qc * 512 + off:(qc + 1) * 512],
                                start=True, stop=True)
                        w0, w1 = offs[0][4], offs[1][4]
                        if w0 == 512:
                            nc.scalar.activation(pexp[:, 0:512 + w1],
                                                 stp[:128, 0:512 + w1],
                                                 AF.Exp, scale=0.125)
                        else:
                            nc.scalar.activation(pexp[:, 0:w0],
                                                 stp[:128, 0:w0],
                                                 AF.Exp, scale=0.125)
                            nc.scalar.activation(pexp[:, 512:512 + w1],
                                                 stp[:128, 512:512 + w1],
                                                 AF.Exp, scale=0.125)
                        for sl, kb, j, off, W in offs:
                            if j >= 0:
                                nc.vector.tensor_mul(
                                    pexp[:, sl * 512:sl * 512 + 128],
                                    pexp[:, sl * 512:sl * 512 + 128], tri[:])
                        for sl, kb, j, off, W in offs:
                            nc.tensor.matmul(
                                ots_[h][:D + 1, off:512],
                                Vall[kb][:, h * (D + 1):(h + 1) * (D + 1)],
                                pexp[:, sl * 512:sl * 512 + W],
                                start=(kb == 0), stop=(kb == nkb - 1))
                # transpose [65,128] blocks back to q-major (f32, keeps l
                # exact): cols 0..63 = O rows, col 64 = l; then C = O / l
                for h in heads:
                    ot = ots_[h]
                    ots = sstr.tile([D + 1, 512], F32, tag='ots')
                    nc.vector.tensor_copy(ots[:], ot[:D + 1, :])
                    for qb in range(4):
                        tp = psC.tile([128, 128], F32, tag='tp')
                        nc.tensor.transpose(tp[:, :D + 1],
                                            ots[:, qb * 128:(qb + 1) * 128],
                                            identf[:D + 1, :D + 1])
                        rl = sstr.tile([128, 1], F32, tag='rl')
                        nc.vector.reciprocal(rl[:], tp[:, D:D + 1])
                        nc.vector.tensor_scalar_mul(
                            C[qc * 4 + qb][:, h * D:(h + 1) * D],
                            tp[:, 0:D], rl[:])
        apool.__exit__(None, None, None)

        # ---- WOT (bf16): host supplies W_O^T; gpsimd DMA casts ----------
        WOTall = bigT.tile([128, 8 * E], BF, tag='bigT', name='WOTall')
        for sc in range(8):
            nc.gpsimd.dma_start(WOTall[:, sc * E:(sc + 1) * E],
                                wo[sc * 128:(sc + 1) * 128, :])
        WOT = [WOTall[:, sc * E:(sc + 1) * E] for sc in range(8)]
        with tc.tile_pool(name='ostr', bufs=3) as ostr:
            # ---- output projection ---------------------------------------
            for it in range(8):
                ps = psA.tile([128, 512], F32, tag='mm')
                for sc in range(8):
                    nc.tensor.matmul(ps[:],
                                     WOT[sc][:, it * 128:(it + 1) * 128],
                                     C[sc][:],
                                     start=(sc == 0), stop=(sc == 7))
                ys = ostr.tile([128, NO], F32, tag='ys')
                nc.vector.tensor_copy(ys[:], ps[:])
                nc.sync.dma_start(out[it * 128:(it + 1) * 128, :], ys[:])


_NC_CACHE = None


def _get_nc():
    global _NC_CACHE
    if _NC_CACHE is None:
        _NC_CACHE = build_nc()
    return _NC_CACHE


def make_in_maps(x, Wq, Wk, Wv, W_O):
    import ml_dtypes
    bf = ml_dtypes.bfloat16
    x = np.asarray(x, np.float32)
    xT_by_b = [np.ascontiguousarray(x[b].T.astype(bf)) for b in range(4)]
    W_O = np.ascontiguousarray(np.asarray(W_O, np.float32).T.astype(bf))
    in_maps = []
    for c in range(8):
        b, g = c // 2, c % 2
        hsl = slice(HC * g, HC * g + HC)
        in_maps.append({
            'xb': xT_by_b[b],
            'wq': np.ascontiguousarray(
                np.asarray(Wq, np.float32)[hsl].transpose(1, 0, 2)
                .reshape(E, HC * D).astype(bf)),
            'wk': np.ascontiguousarray(
                np.asarray(Wk, np.float32)[hsl].transpose(1, 0, 2)
                .reshape(E, HC * D).astype(bf)),
            'wv': np.ascontiguousarray(
                np.asarray(Wv, np.float32)[hsl].transpose(1, 0, 2)
                .reshape(E, HC * D).astype(bf)),
            'wo': W_O,
        })
    return in_maps


def kernel(x, Wq, Wk, Wv, W_O):
    from concourse.bass_utils import run_bass_kernel_spmd
    nc = _get_nc()
    in_maps = make_in_maps(x, Wq, Wk, Wv, W_O)
    res = run_bass_kernel_spmd(nc, in_maps, list(range(8)))
    full = np.empty((4, E, E), np.float32)
    for c in range(8):
        b, g = c // 2, c % 2
        full[b, :, NO * g:NO * g + NO] = res.results[c]['out']
    return full



# revision 4
# speedup vs baseline: 1.1193x; 1.1193x over previous
"""Trainium2 Bass kernel v2 for nn_MultiHeadAttention_8667244003725.

B=4, S=1024, E=1024, H=16, D=64.  Same sharding as baseline:
8 cores = 4 batches x 2 head-groups; core c -> out[b, :, 512g:512g+512],
b=c//2, g=c%2.  No collectives.

v2 changes vs baseline (173us):
- scores row-tiled: both heads of a pair run CONCURRENTLY on the PE
  (K=64 contractions at tile rows 0-63 / 64-127) into one [128,1024] slab.
- one exp per (pair,qc,kb) covering both heads via strided AP; one DVE
  tri-mask per diagonal pair (broadcast AP).
- QKV jobs for later pairs emitted as filler units inside the attention
  chain so the PE never idles on exp -> HAM stays at 2.4 GHz.
- transposes: f32 [65,128] blocks into a shared psum ring; normalize via
  one reciprocal [128,4] + one broadcast tensor_mul per (head, qc).
- weight DMAs off the scalar queue; V/Q/K psum drains on gpsimd/scalar.
"""

import sys

if '/opt/trn_rl_repo' not in sys.path:
    sys.path.insert(0, '/opt/trn_rl_repo')

import numpy as np

import concourse.bass as bass
import concourse.mybir as mybir
import concourse.tile as tile
from concourse.masks import make_identity

F32 = mybir.dt.float32
BF = mybir.dt.bfloat16
AF = mybir.ActivationFunctionType
MUL = mybir.AluOpType.mult

S = 1024
E = 1024
D = 64
HC = 8            # heads per core
NO = 512          # output columns per core


def _split_sync_waits(nc, limit=1):
    """Walrus here rejects >1 sem-wait per instruction; hoist extras onto
    same-engine no-ops."""
    n = 0
    for f in nc.m.functions:
        for bb in f.blocks:
            out = []
            for ins in bb.instructions:
                si = ins.sync_info
                waits = list(si.on_wait) if si is not None else []
                if len(waits) > limit:
                    excess, keep = waits[:-limit], waits[-limit:]
                    for i in range(0, len(excess), limit):
                        grp = excess[i:i + limit]
                        n += 1
                        out.append(mybir.InstNoOp(
                            name=f'I-synsplit-{n}', ins=[], outs=[],
                            engine=ins.engine,
                            sync_info=mybir.SyncInfo(on_wait=list(grp),
                                                     on_update=[])))
                    si.on_wait = keep
                out.append(ins)
            bb.instructions = out
    return n


def build_nc(split_waits=True, debug=False):
    nc = bass.Bass()
    xb = nc.dram_tensor('xb', [E, S], BF, kind='ExternalInput')   # x[b]^T
    wq = nc.dram_tensor('wq', [E, HC * D], BF, kind='ExternalInput')
    wk = nc.dram_tensor('wk', [E, HC * D], BF, kind='ExternalInput')
    wv = nc.dram_tensor('wv', [E, HC * D], BF, kind='ExternalInput')
    wo = nc.dram_tensor('wo', [E, E], BF, kind='ExternalInput')   # W_O^T
    out = nc.dram_tensor('out', [E, NO], F32, kind='ExternalOutput')
    dbg = None
    if debug:
        dbg = {
            'dC': nc.dram_tensor('dC', [128, 8 * NO], BF,
                                 kind='ExternalOutput'),
            'dQ': nc.dram_tensor('dQ', [4 * 128, S], BF,
                                 kind='ExternalOutput'),
            'dK': nc.dram_tensor('dK', [4 * 128, S], BF,
                                 kind='ExternalOutput'),
            'dV': nc.dram_tensor('dV', [8 * 128, HC * (D + 1)], BF,
                                 kind='ExternalOutput'),
        }

    with tile.TileContext(nc) as tc:
        _emit(nc, tc, xb, wq, wk, wv, wo, out, dbg=dbg)
    if split_waits:
        _split_sync_waits(nc)
    return nc


def _copier(eng):
    """Uniform copy callable: scalar uses activation-Copy, DVE tensor_copy.
    (GPSIMD cannot read PSUM on trn2.)"""
    if hasattr(eng, 'tensor_copy'):
        return eng.tensor_copy
    return eng.copy


def _emit(nc, tc, xb, wq, wk, wv, wo, out, dbg=None):
    with (
        tc.tile_pool(name='const', bufs=1) as constp,
        tc.tile_pool(name='big', bufs=1) as bigp,      # xT, weights, WOT, C
        tc.tile_pool(name='qk', bufs=1) as qkp,
        tc.tile_pool(name='vall', bufs=1) as vallp,
        tc.tile_pool(name='pexp', bufs=4) as pexpp,
        tc.tile_pool(name='otsb', bufs=2) as otsbp,
        tc.tile_pool(name='rl', bufs=2) as rlp,
        tc.tile_pool(name='ys', bufs=2) as ysp,
        tc.tile_pool(name='sc', bufs=3, space='PSUM') as scp,   # 6 banks
        tc.tile_pool(name='av', bufs=2, space='PSUM') as avp,   # 2 banks
    ):
        # ---- constants --------------------------------------------------
        identf = constp.tile([128, 128], F32, tag='identf')
        make_identity(nc, identf[:])
        ones8 = constp.tile([128, 8], BF, tag='ones8')
        nc.gpsimd.memset(ones8[:], 1.0)
        # tri[k, q] = 1 where q >= k else 0 (multiplicative causal mask)
        tri = constp.tile([128, 128], BF, tag='tri')
        nc.gpsimd.memset(tri[:], 1.0)
        nc.gpsimd.affine_select(
            out=tri[:], in_=tri[:], compare_op=mybir.AluOpType.is_ge,
            fill=0.0, base=0, channel_multiplier=-1, pattern=[[1, 128]])
        # warm the ACT exp table first thing on the scalar queue
        warm = constp.tile([1, 2], F32, tag='warm')
        nc.scalar.activation(warm[:], ones8[0:1, 0:2], AF.Exp, scale=0.125)

        # ---- input DMAs -------------------------------------------------
        # sync queue: wq then x[0:4];  scalar queue: x[4:8] then wk;
        # gpsimd queue: wv (wo later, before proj).
        wpq = bigp.tile([128, 8 * HC * D], BF, tag='wq', name='wqall')
        wpk = bigp.tile([128, 8 * HC * D], BF, tag='wk', name='wkall')
        wpv = bigp.tile([128, 8 * HC * D], BF, tag='wv', name='wvall')
        xTall = bigp.tile([128, 8 * S], BF, tag='xT', name='xTall')
        for ec in range(8):
            nc.sync.dma_start(wpq[:, ec * 512:(ec + 1) * 512],
                              wq[ec * 128:(ec + 1) * 128, :])
        for ec in range(4, 8):
            nc.scalar.dma_start(xTall[:, ec * S:(ec + 1) * S],
                                xb[ec * 128:(ec + 1) * 128, :])
        for ec in range(4):
            nc.sync.dma_start(xTall[:, ec * S:(ec + 1) * S],
                              xb[ec * 128:(ec + 1) * 128, :])
        for ec in range(8):
            nc.scalar.dma_start(wpk[:, ec * 512:(ec + 1) * 512],
                                wk[ec * 128:(ec + 1) * 128, :])
            nc.gpsimd.dma_start(wpv[:, ec * 512:(ec + 1) * 512],
                                wv[ec * 128:(ec + 1) * 128, :])
        xT = [xTall[:, ec * S:(ec + 1) * S] for ec in range(8)]
        wqt = [wpq[:, ec * 512:(ec + 1) * 512] for ec in range(8)]
        wkt = [wpk[:, ec * 512:(ec + 1) * 512] for ec in range(8)]
        wvt = [wpv[:, ec * 512:(ec + 1) * 512] for ec in range(8)]

        # ---- persistent SBUF --------------------------------------------
        QT2 = [qkp.tile([128, S], BF, tag=f'q{p}', name=f'QT2_{p}')
               for p in range(4)]
        KT2 = [qkp.tile([128, S], BF, tag=f'k{p}', name=f'KT2_{p}')
               for p in range(4)]
        Vall = [vallp.tile([128, HC * (D + 1)], BF, tag=f'v{st}',
                           name=f'Vall{st}') for st in range(8)]
        Call = bigp.tile([128, 8 * NO], BF, tag='C', name='Call')
        WOTall = bigp.tile([128, 8 * E], BF, tag='WOT', name='WOTall')
        WOT = [WOTall[:, i * E:(i + 1) * E] for i in range(8)]

        # ---- job generators (each yield = ~0.4-0.9us of PE work) --------
        def gen_qk(p, wt, dst, copy_eng, ec_order=None):
            ecs = ec_order or list(range(8))
            ps = scp.tile([128, 1024], F32, tag='sc', name=f'qk_{p}_{wt is wkt}')
            for i, ec in enumerate(ecs):
                for s2 in range(2):
                    nc.tensor.matmul(
                        ps[:, s2 * 512:(s2 + 1) * 512],
                        wt[ec][:, p * 128:(p + 1) * 128],
                        xT[ec][:, s2 * 512:(s2 + 1) * 512],
                        start=(i == 0), stop=(i == 7))
                if i % 2 == 1 and i < 7:
                    yield
            copy_eng(dst[:], ps[:])
            yield

        def gen_v(st):
            ps = scp.tile([128, 1024], F32, tag='sc', name=f'v_{st}')
            for i in range(8):
                nc.tensor.matmul(ps[:, 0:512],
                                 xT[i][:, st * 128:(st + 1) * 128],
                                 wvt[i], start=(i == 0), stop=(i == 7))
                if i % 2 == 1 and i < 7:
                    yield
            v3 = Vall[st][:].rearrange('p (h d) -> p h d', h=HC)
            cp = _copier(nc.scalar if st < 4 else nc.vector)
            cp(v3[:, :, 0:D],
               ps[:, 0:512].rearrange('p (h d) -> p h d', h=HC))
            cp(v3[:, :, D:D + 1],
               ones8[:].rearrange('p (h o) -> p h o', o=1))
            yield

        def drain(g):
            for _ in g:
                pass

        from collections import deque
        fillers = deque()

        def fill(n=1):
            k = 0
            while k < n and fillers:
                try:
                    next(fillers[0])
                    k += 1
                except StopIteration:
                    fillers.popleft()

        # ---- prologue: Q0 K0 V0..7 Q1 K1 (PE-dense, warms HAM) ----------
        # Fillers must be emitted at least one attention window before their
        # consumer (forward deps through the filler stream produced stale
        # reads on HW), so V goes fully in the prologue and QK(p) fills
        # during pair p-2's window.
        drain(gen_qk(0, wqt, QT2[0], _copier(nc.scalar),
                     ec_order=[4, 5, 6, 7, 0, 1, 2, 3]))
        drain(gen_qk(0, wkt, KT2[0], _copier(nc.scalar)))
        for st in range(8):
            drain(gen_v(st))
        drain(gen_qk(1, wqt, QT2[1], _copier(nc.scalar)))
        drain(gen_qk(1, wkt, KT2[1], _copier(nc.scalar)))

        # ---- filler for the attention phase -----------------------------
        fillers.append(gen_qk(2, wqt, QT2[2], _copier(nc.vector)))
        fillers.append(gen_qk(2, wkt, KT2[2], _copier(nc.vector)))
        per_pair_fillers = {
            1: [gen_qk(3, wqt, QT2[3], _copier(nc.vector)),
                gen_qk(3, wkt, KT2[3], _copier(nc.vector))],
        }

        # ---- attention ---------------------------------------------------
        C3 = Call[:].rearrange('p (st c) -> p st c', st=8)
        tri_bc = tri[:, None, :].broadcast_to((128, 2, 128))

        def emit_scores(p, qc, kb):
            """Row-tiled pair of score matmuls + exp + causal mask.
            Returns the pexp tile."""
            j = kb - 4 * qc
            off = 128 * j if j >= 0 else 0
            stp = scp.tile([128, 1024], F32, tag='sc',
                           name=f'stp_{p}_{qc}_{kb}')
            for hh in range(2):
                nc.tensor.matmul(
                    stp[:, hh * 512 + off:(hh + 1) * 512],
                    KT2[p][64 * hh:64 * hh + 64, kb * 128:(kb + 1) * 128],
                    QT2[p][64 * hh:64 * hh + 64,
                           qc * 512 + off:(qc + 1) * 512],
                    start=True, stop=True)
            pexp = pexpp.tile([128, 1024], BF, tag='pexp',
                              name=f'pexp_{p}_{qc}_{kb}')
            stp3 = stp[:].rearrange('p (two c) -> p two c', two=2)
            pexp3 = pexp[:].rearrange('p (two c) -> p two c', two=2)
            nc.scalar.activation(pexp3[:, :, off:512], stp3[:, :, off:512],
                                 AF.Exp, scale=0.125)
            if j >= 0:
                nc.vector.tensor_mul(pexp3[:, :, off:off + 128],
                                     pexp3[:, :, off:off + 128], tri_bc)
            return pexp, off

        for p in range(4):
            for g in per_pair_fillers.get(p, []):
                fillers.append(g)
            if p >= 2:
                while fillers:   # QK(p) leftovers must land before pair p
                    fill(1)
            for qc in range(2):
                kbs = list(range(4 * qc + 4))
                n = len(kbs)
                ots = [avp.tile([128, 512], F32, tag='av',
                                name=f'ot_{p}_{qc}_{hh}') for hh in range(2)]
                pend = {}
                pend[0] = emit_scores(p, qc, 0)
                if n > 1:
                    pend[1] = emit_scores(p, qc, 1)
                for t in range(n):
                    if t % 2 == 1:
                        fill(1)
                    pexp, off = pend.pop(t)
                    for hh in range(2):
                        nc.tensor.matmul(
                            ots[hh][:D + 1, off:512],
                            Vall[t][:, (2 * p + hh) * (D + 1):
                                    (2 * p + hh + 1) * (D + 1)],
                            pexp[:, hh * 512 + off:(hh + 1) * 512],
                            start=(t == 0), stop=(t == n - 1))
                    if t + 2 < n:
                        pend[t + 2] = emit_scores(p, qc, t + 2)
                # transpose + normalize -> C
                for hh in range(2):
                    h = 2 * p + hh
                    otsb = otsbp.tile([128, 512], F32, tag='otsb',
                                      name=f'otsb_{p}_{qc}_{hh}')
                    if hh == 0:
                        nc.scalar.copy(otsb[:D + 1, :], ots[hh][:D + 1, :])
                    else:
                        nc.vector.tensor_copy(otsb[:D + 1, :],
                                              ots[hh][:D + 1, :])
                    fill(1)
                    tp = scp.tile([128, 1024], F32, tag='sc',
                                  name=f'tp_{p}_{qc}_{hh}')
                    for qb in range(4):
                        nc.tensor.transpose(
                            tp[:, qb * 65:qb * 65 + 65],
                            otsb[:D + 1, qb * 128:(qb + 1) * 128],
                            identf[:D + 1, :D + 1])
                    tp3 = tp[:, 0:260].rearrange('p (qb c) -> p qb c', qb=4)
                    rl = rlp.tile([128, 4], F32, tag='rl',
                                  name=f'rl_{p}_{qc}_{hh}')
                    nc.vector.reciprocal(rl[:], tp3[:, :, D])
                    nc.vector.tensor_mul(
                        C3[:, qc * 4:(qc + 1) * 4, h * D:(h + 1) * D],
                        tp3[:, :, 0:D],
                        rl[:, :, None].broadcast_to((128, 4, D)))
            if p == 1:
                # WOT lands on the gpsimd queue well before proj needs it
                for i in range(8):
                    nc.gpsimd.dma_start(WOTall[:, i * E:(i + 1) * E],
                                        wo[i * 128:(i + 1) * 128, :])

        while fillers:
            fill(1)

        if dbg is not None:
            nc.sync.dma_start(dbg['dC'][:, :], Call[:])
            for p4 in range(4):
                nc.sync.dma_start(dbg['dQ'][p4 * 128:(p4 + 1) * 128, :],
                                  QT2[p4][:])
                nc.sync.dma_start(dbg['dK'][p4 * 128:(p4 + 1) * 128, :],
                                  KT2[p4][:])
            for st in range(8):
                nc.sync.dma_start(dbg['dV'][st * 128:(st + 1) * 128, :],
                                  Vall[st][:])

        # ---- output projection ------------------------------------------
        for it in range(8):
            ps = scp.tile([128, 1024], F32, tag='sc', name=f'proj_{it}')
            for i in range(8):
                nc.tensor.matmul(ps[:, 0:512],
                                 WOT[i][:, it * 128:(it + 1) * 128],
                                 Call[:, i * 512:(i + 1) * 512],
                                 start=(i == 0), stop=(i == 7))
            ys = ysp.tile([128, NO], F32, tag='ys', name=f'ys_{it}')
            nc.scalar.copy(ys[:], ps[:, 0:512])
            nc.sync.dma_start(out[it * 128:(it + 1) * 128, :], ys[:])


_NC_CACHE = None


def _get_nc():
    global _NC_CACHE
    if _NC_CACHE is None:
        _NC_CACHE = build_nc()
    return _NC_CACHE


def make_in_maps(x, Wq, Wk, Wv, W_O):
    import ml_dtypes
    bf = ml_dtypes.bfloat16
    x = np.asarray(x, np.float32)
    xT_by_b = [np.ascontiguousarray(x[b].T.astype(bf)) for b in range(4)]
    W_O = np.ascontiguousarray(np.asarray(W_O, np.float32).T.astype(bf))
    in_maps = []
    for c in range(8):
        b, g = c // 2, c % 2
        hsl = slice(HC * g, HC * g + HC)
        in_maps.append({
            'xb': xT_by_b[b],
            'wq': np.ascontiguousarray(
                np.asarray(Wq, np.float32)[hsl].transpose(1, 0, 2)
                .reshape(E, HC * D).astype(bf)),
            'wk': np.ascontiguousarray(
                np.asarray(Wk, np.float32)[hsl].transpose(1, 0, 2)
                .reshape(E, HC * D).astype(bf)),
            'wv': np.ascontiguousarray(
                np.asarray(Wv, np.float32)[hsl].transpose(1, 0, 2)
                .reshape(E, HC * D).astype(bf)),
            'wo': W_O,
        })
    return in_maps


def kernel(x, Wq, Wk, Wv, W_O):
    from concourse.bass_utils import run_bass_kernel_spmd
    nc = _get_nc()
    in_maps = make_in_maps(x, Wq, Wk, Wv, W_O)
    res = run_bass_kernel_spmd(nc, in_maps, list(range(8)))
    full = np.empty((4, E, E), np.float32)
    for c in range(8):
        b, g = c // 2, c % 2
        full[b, :, NO * g:NO * g + NO] = res.results[c]['out']
    return full
